# revision 2
# baseline (speedup 1.0000x reference)
"""Trainium2 Bass kernel for nn_MinimalQuantumCell.

Computes, for full inputs
    x         [4096, 256]  f32
    h_quantum [4096, 512, 16] f32
    W_quantum [256, 512, 16]  f32
the pair
    output    [4096, 512]      = mean_s tanh(x @ W + 0.9 h)
    new_state [4096, 512, 16]  = tanh(x @ W + 0.9 h)

Strategy: data-parallel over the batch axis across 8 NeuronCores
(512 rows each), W replicated.  Per core:
  - x^T (pre-transposed on host) and W live in SBUF, streamed in once.
  - For each [128 batch x 512 (hid*s)] tile: PSUM accumulates x@W over
    two K=128 matmuls; VectorE adds 0.9*h (pre-scaled on host) in exact
    fp32; ScalarE applies tanh; VectorE reduces groups of 16 states for
    the mean; DMA streams h in / state out.
"""

import numpy as np
from contextlib import ExitStack

BATCH, IN_DIM, HID, STATES = 4096, 256, 512, 16
N_CORES = 8
P = 128          # SBUF partitions
N_TILE = 512     # free-dim tile (one PSUM bank of f32)

_CACHE = {}

MM_DTYPE = "float32r"   # "float32r" (fast, ~1e-3 err) or "float32" (exact)


def _build_nc(mm_dtype=MM_DTYPE, n_cores=N_CORES):
    import concourse.tile as tile
    from concourse import bacc, mybir

    B_LOC = BATCH // n_cores          # 512 batch rows per core
    N_TOT = HID * STATES              # 8192
    KT = IN_DIM // P                  # 2 k-tiles
    MT = B_LOC // P                   # 4 m-tiles
    NT = N_TOT // N_TILE              # 16 n-tiles
    GP = N_TILE // STATES             # 32 hid groups per n-tile

    f32 = mybir.dt.float32
    mmdt = getattr(mybir.dt, mm_dtype)

    def r(ap):
        # view f32 DRAM/SBUF data as the matmul dtype (no conversion for
        # f32; pure reinterpret for f32r since tiles are declared f32r)
        return ap if mm_dtype == "float32" else ap.bitcast(mmdt)

    nc = bacc.Bacc(
        "TRN2", target_bir_lowering=False, debug=False, num_devices=n_cores
    )
    xT = nc.dram_tensor("xT", [IN_DIM, B_LOC], f32, kind="ExternalInput").ap()
    w = nc.dram_tensor("w", [IN_DIM, N_TOT], f32, kind="ExternalInput").ap()
    h = nc.dram_tensor("h", [B_LOC, N_TOT], f32, kind="ExternalInput").ap()
    state = nc.dram_tensor("state", [B_LOC, N_TOT], f32, kind="ExternalOutput").ap()
    mean = nc.dram_tensor("mean", [B_LOC, HID], f32, kind="ExternalOutput").ap()

    with tile.TileContext(nc) as tc, ExitStack() as ctx:
        singles = ctx.enter_context(tc.tile_pool(name="singles", bufs=1))
        hpool = ctx.enter_context(tc.tile_pool(name="hin", bufs=6))
        spool = ctx.enter_context(tc.tile_pool(name="stout", bufs=6))
        mpool = ctx.enter_context(tc.tile_pool(name="meanacc", bufs=2))
        pspool = ctx.enter_context(tc.tile_pool(name="psum", bufs=6, space="PSUM"))

        xT_sb = singles.tile([P, KT, B_LOC], mmdt)
        for k in range(KT):
            nc.sync.dma_start(xT_sb[:, k, :], r(xT[k * P:(k + 1) * P, :]))
        w_sb = singles.tile([P, KT, N_TOT], mmdt)
        for k in range(KT):
            for n in range(NT):
                nsl = slice(n * N_TILE, (n + 1) * N_TILE)
                nc.sync.dma_start(w_sb[:, k, nsl], r(w[k * P:(k + 1) * P, nsl]))

        for m in range(MT):
            msl = slice(m * P, (m + 1) * P)
            macc = mpool.tile([P, HID], f32)
            for n in range(NT):
                nsl = slice(n * N_TILE, (n + 1) * N_TILE)
                h_t = hpool.tile([P, N_TILE], f32)
                nc.sync.dma_start(h_t[:], h[msl, nsl])

                ps = pspool.tile([P, N_TILE], f32)
                for k in range(KT):
                    nc.tensor.matmul(
                        ps[:],
                        xT_sb[:, k, msl],
                        w_sb[:, k, nsl],
                        start=(k == 0),
                        stop=(k == KT - 1),
                    )

                st = spool.tile([P, N_TILE], f32)
                # exact fp32: st = psum + 0.9*h   (h pre-scaled on host)
                nc.vector.tensor_tensor(
                    st[:], ps[:], h_t[:], op=mybir.AluOpType.add
                )
                nc.scalar.activation(
                    st[:], st[:], mybir.ActivationFunctionType.Tanh
                )
                nc.vector.tensor_reduce(
                    macc[:, n * GP:(n + 1) * GP],
                    st[:].rearrange("p (g s) -> p g s", s=STATES),
                    axis=mybir.AxisListType.X,
                    op=mybir.AluOpType.add,
                )
                nc.sync.dma_start(state[msl, nsl], st[:])

            mout = mpool.tile([P, HID], f32)
            nc.vector.tensor_scalar_mul(mout[:], macc[:], 1.0 / STATES)
            nc.sync.dma_start(mean[msl, :], mout[:])

    nc.compile()
    return nc


def _get_nc():
    if "nc" not in _CACHE:
        _CACHE["nc"] = _build_nc()
    return _CACHE["nc"]


def _shard_inputs(x, h, W):
    B_LOC = BATCH // N_CORES
    w2 = np.ascontiguousarray(W.reshape(IN_DIM, HID * STATES))
    h2 = (h.reshape(BATCH, HID * STATES) * np.float32(0.9))
    in_maps = []
    for c in range(N_CORES):
        sl = slice(c * B_LOC, (c + 1) * B_LOC)
        in_maps.append({
            "xT": np.ascontiguousarray(x[sl].T),
            "w": w2,
            "h": np.ascontiguousarray(h2[sl]),
        })
    return in_maps


def kernel(x, h_quantum, W_quantum, _nc=None, _run_kwargs=None):
    from concourse.bass_utils import run_bass_kernel_spmd

    x = np.asarray(x, dtype=np.float32)
    h = np.asarray(h_quantum, dtype=np.float32)
    W = np.asarray(W_quantum, dtype=np.float32)

    nc = _nc if _nc is not None else _get_nc()
    in_maps = _shard_inputs(x, h, W)
    res = run_bass_kernel_spmd(
        nc, in_maps, core_ids=list(range(N_CORES)), **(_run_kwargs or {})
    )
    outs = res.results
    state = np.concatenate(
        [outs[c]["state"] for c in range(N_CORES)], axis=0
    ).reshape(BATCH, HID, STATES)
    mean = np.concatenate([outs[c]["mean"] for c in range(N_CORES)], axis=0)
    if _run_kwargs:
        _CACHE["last_results"] = res
    return mean.astype(np.float32, copy=False), state.astype(np.float32, copy=False)


# revision 4
# speedup vs baseline: 1.0968x; 1.0968x over previous
"""Trainium2 Bass kernel for nn_MinimalQuantumCell.

Computes, for full inputs
    x         [4096, 256]  f32
    h_quantum [4096, 512, 16] f32
    W_quantum [256, 512, 16]  f32
the pair
    output    [4096, 512]      = mean_s tanh(x @ W + 0.9 h)
    new_state [4096, 512, 16]  = tanh(x @ W + 0.9 h)

Strategy: data-parallel over the batch axis across 8 NeuronCores
(512 rows each), W replicated.  Per core:
  - x^T (pre-transposed on host) and W live in SBUF, streamed in once.
  - For each [128 batch x 512 (hid*s)] tile: PSUM accumulates x@W over
    two K=128 matmuls; VectorE adds 0.9*h (pre-scaled on host) in exact
    fp32; ScalarE applies tanh; VectorE reduces groups of 16 states for
    the mean; DMA streams h in / state out.
"""

import numpy as np
from contextlib import ExitStack

BATCH, IN_DIM, HID, STATES = 4096, 256, 512, 16
N_CORES = 8
P = 128          # SBUF partitions
N_TILE = 512     # free-dim tile (one PSUM bank of f32)

_CACHE = {}

MM_DTYPE = "float32r"   # "float32r" (fast, ~1e-3 err) or "float32" (exact)


def _build_nc(mm_dtype=MM_DTYPE, n_cores=N_CORES):
    import concourse.tile as tile
    from concourse import bacc, mybir

    B_LOC = BATCH // n_cores          # 512 batch rows per core
    N_TOT = HID * STATES              # 8192
    KT = IN_DIM // P                  # 2 k-tiles
    MT = B_LOC // P                   # 4 m-tiles
    NT = N_TOT // N_TILE              # 16 n-tiles
    GP = N_TILE // STATES             # 32 hid groups per n-tile

    f32 = mybir.dt.float32
    mmdt = getattr(mybir.dt, mm_dtype)

    def r(ap):
        # view f32 DRAM/SBUF data as the matmul dtype (no conversion for
        # f32; pure reinterpret for f32r since tiles are declared f32r)
        return ap if mm_dtype == "float32" else ap.bitcast(mmdt)

    nc = bacc.Bacc(
        "TRN2", target_bir_lowering=False, debug=False, num_devices=n_cores
    )
    xT = nc.dram_tensor("xT", [IN_DIM, B_LOC], f32, kind="ExternalInput").ap()
    w = nc.dram_tensor("w", [IN_DIM, N_TOT], f32, kind="ExternalInput").ap()
    h = nc.dram_tensor("h", [B_LOC, N_TOT], f32, kind="ExternalInput").ap()
    state = nc.dram_tensor("state", [B_LOC, N_TOT], f32, kind="ExternalOutput").ap()
    mean = nc.dram_tensor("mean", [B_LOC, HID], f32, kind="ExternalOutput").ap()

    with tile.TileContext(nc) as tc, ExitStack() as ctx:
        singles = ctx.enter_context(tc.tile_pool(name="singles", bufs=1))
        hpool = ctx.enter_context(tc.tile_pool(name="hin", bufs=8))
        spool = ctx.enter_context(tc.tile_pool(name="stout", bufs=8))
        mpool = ctx.enter_context(tc.tile_pool(name="meanacc", bufs=2))
        pspool = ctx.enter_context(tc.tile_pool(name="psum", bufs=8, space="PSUM"))

        xT_sb = singles.tile([P, KT, B_LOC], mmdt)
        for k in range(KT):
            nc.gpsimd.dma_start(xT_sb[:, k, :], r(xT[k * P:(k + 1) * P, :]))
        w_sb = singles.tile([P, KT, N_TOT], mmdt)
        for n in range(NT):
            for k in range(KT):
                nsl = slice(n * N_TILE, (n + 1) * N_TILE)
                nc.gpsimd.dma_start(w_sb[:, k, nsl], r(w[k * P:(k + 1) * P, nsl]))

        for m in range(MT):
            msl = slice(m * P, (m + 1) * P)
            macc = mpool.tile([P, HID], f32)
            for n in range(NT):
                nsl = slice(n * N_TILE, (n + 1) * N_TILE)
                h_t = hpool.tile([P, N_TILE], f32)
                nc.sync.dma_start(h_t[:], h[msl, nsl])

                ps = pspool.tile([P, N_TILE], f32)
                for k in range(KT):
                    nc.tensor.matmul(
                        ps[:],
                        xT_sb[:, k, msl],
                        w_sb[:, k, nsl],
                        start=(k == 0),
                        stop=(k == KT - 1),
                    )

                st = spool.tile([P, N_TILE], f32)
                # exact fp32: st = psum + 0.9*h   (h pre-scaled on host)
                nc.vector.tensor_tensor(
                    st[:], ps[:], h_t[:], op=mybir.AluOpType.add
                )
                nc.scalar.activation(
                    st[:], st[:], mybir.ActivationFunctionType.Tanh
                )
                nc.vector.tensor_reduce(
                    macc[:, n * GP:(n + 1) * GP],
                    st[:].rearrange("p (g s) -> p g s", s=STATES),
                    axis=mybir.AxisListType.X,
                    op=mybir.AluOpType.add,
                )
                nc.scalar.dma_start(state[msl, nsl], st[:])

            mout = mpool.tile([P, HID], f32)
            nc.vector.tensor_scalar_mul(mout[:], macc[:], 1.0 / STATES)
            nc.gpsimd.dma_start(mean[msl, :], mout[:])

    nc.compile()
    return nc


def _get_nc():
    if "nc" not in _CACHE:
        _CACHE["nc"] = _build_nc()
    return _CACHE["nc"]


def _shard_inputs(x, h, W):
    B_LOC = BATCH // N_CORES
    w2 = np.ascontiguousarray(W.reshape(IN_DIM, HID * STATES))
    h2 = (h.reshape(BATCH, HID * STATES) * np.float32(0.9))
    in_maps = []
    for c in range(N_CORES):
        sl = slice(c * B_LOC, (c + 1) * B_LOC)
        in_maps.append({
            "xT": np.ascontiguousarray(x[sl].T),
            "w": w2,
            "h": np.ascontiguousarray(h2[sl]),
        })
    return in_maps


def kernel(x, h_quantum, W_quantum, _nc=None, _run_kwargs=None):
    from concourse.bass_utils import run_bass_kernel_spmd

    x = np.asarray(x, dtype=np.float32)
    h = np.asarray(h_quantum, dtype=np.float32)
    W = np.asarray(W_quantum, dtype=np.float32)

    nc = _nc if _nc is not None else _get_nc()
    in_maps = _shard_inputs(x, h, W)
    res = run_bass_kernel_spmd(
        nc, in_maps, core_ids=list(range(N_CORES)), **(_run_kwargs or {})
    )
    outs = res.results
    state = np.concatenate(
        [outs[c]["state"] for c in range(N_CORES)], axis=0
    ).reshape(BATCH, HID, STATES)
    mean = np.concatenate([outs[c]["mean"] for c in range(N_CORES)], axis=0)
    if _run_kwargs:
        _CACHE["last_results"] = res
    return mean.astype(np.float32, copy=False), state.astype(np.float32, copy=False)


# revision 7
# speedup vs baseline: 1.1570x; 1.0548x over previous
"""Trainium2 Bass kernel for nn_MinimalQuantumCell.

Computes, for full inputs
    x         [4096, 256]  f32
    h_quantum [4096, 512, 16] f32
    W_quantum [256, 512, 16]  f32
the pair
    output    [4096, 512]      = mean_s tanh(x @ W + 0.9 h)
    new_state [4096, 512, 16]  = tanh(x @ W + 0.9 h)

Strategy: model-parallel over the hid axis across 8 NeuronCores (64 hid
units -> 1024 (hid,s) columns each); x replicated (pre-transposed on
host), W sharded.  All per-core DRAM blocks are contiguous, so every
DMA is a simple linear transfer.  Per core:
  - x^T (4 MB) and the W shard (1 MB) are preloaded to SBUF.
  - For each [128 batch x 1024 col] tile: PSUM accumulates x@W over two
    K=128 matmuls (float32r - full-rate fp32 on the PE); VectorE adds
    0.9*h (pre-scaled on host) in exact fp32; ScalarE applies tanh;
    VectorE reduces groups of 16 states for the mean; DMA streams h
    in (sync queue) / state out (scalar queue).
"""

import numpy as np
from contextlib import ExitStack

BATCH, IN_DIM, HID, STATES = 4096, 256, 512, 16
N_CORES = 8
P = 128          # SBUF partitions
N_TILE = 512     # matmul moving-dim tile (one PSUM bank of f32)

_CACHE = {}

MM_DTYPE = "float32r"   # "float32r" (fast, ~1e-3 err) or "float32" (exact)


def _build_nc(mm_dtype=MM_DTYPE, n_cores=N_CORES):
    import concourse.tile as tile
    from concourse import bacc, mybir

    N_LOC = HID * STATES // n_cores   # 1024 (hid,s) columns per core
    KT = IN_DIM // P                  # 2 k-tiles
    MT = BATCH // P                   # 32 m-tiles
    NT = N_LOC // N_TILE              # 2 n-tiles
    GP = N_TILE // STATES             # 32 hid groups per n-tile
    HID_LOC = HID // n_cores          # 64 hid units per core

    f32 = mybir.dt.float32
    mmdt = getattr(mybir.dt, mm_dtype)

    def r(ap):
        # view f32 DRAM data as the matmul dtype (pure reinterpret; the
        # PE rounds internally for f32r)
        return ap if mm_dtype == "float32" else ap.bitcast(mmdt)

    nc = bacc.Bacc(
        "TRN2", target_bir_lowering=False, debug=False, num_devices=n_cores
    )
    xT = nc.dram_tensor("xT", [IN_DIM, BATCH], f32, kind="ExternalInput").ap()
    w = nc.dram_tensor("w", [IN_DIM, N_LOC], f32, kind="ExternalInput").ap()
    h = nc.dram_tensor("h", [BATCH, N_LOC], f32, kind="ExternalInput").ap()
    state = nc.dram_tensor("state", [BATCH, N_LOC], f32, kind="ExternalOutput").ap()
    mean = nc.dram_tensor("mean", [BATCH, HID_LOC], f32, kind="ExternalOutput").ap()

    with tile.TileContext(nc) as tc, ExitStack() as ctx:
        singles = ctx.enter_context(tc.tile_pool(name="singles", bufs=1))
        hpool = ctx.enter_context(tc.tile_pool(name="hin", bufs=8))
        spool = ctx.enter_context(tc.tile_pool(name="stout", bufs=8))
        mpool = ctx.enter_context(tc.tile_pool(name="meanacc", bufs=4))
        pspool = ctx.enter_context(tc.tile_pool(name="psum", bufs=4, space="PSUM"))

        w_sb = singles.tile([P, KT, N_LOC], mmdt)
        for k in range(KT):
            nc.gpsimd.dma_start(w_sb[:, k, :], r(w[k * P:(k + 1) * P, :]))
        xT_sb = singles.tile([P, KT, BATCH], mmdt)
        for k in range(KT):
            # split the 2MB transfer so compute can start early
            for c in range(4):
                csl = slice(c * (BATCH // 4), (c + 1) * (BATCH // 4))
                nc.gpsimd.dma_start(xT_sb[:, k, csl], r(xT[k * P:(k + 1) * P, csl]))

        for m in range(MT):
            msl = slice(m * P, (m + 1) * P)
            h_t = hpool.tile([P, N_LOC], f32)
            nc.sync.dma_start(h_t[:], h[msl, :])

            macc = mpool.tile([P, HID_LOC], f32)
            pss = [
                pspool.tile([P, N_TILE], f32, name=f"ps{n}", tag=f"ps{n}")
                for n in range(NT)
            ]
            for k in range(KT):
                for n in range(NT):
                    nsl = slice(n * N_TILE, (n + 1) * N_TILE)
                    nc.tensor.matmul(
                        pss[n][:],
                        xT_sb[:, k, msl],
                        w_sb[:, k, nsl],
                        start=(k == 0),
                        stop=(k == KT - 1),
                    )
            st = spool.tile([P, N_LOC], f32)
            for n in range(NT):
                nsl = slice(n * N_TILE, (n + 1) * N_TILE)
                # exact fp32: st = psum + 0.9*h   (h pre-scaled on host)
                nc.vector.tensor_tensor(
                    st[:, nsl], pss[n][:], h_t[:, nsl], op=mybir.AluOpType.add
                )
                nc.scalar.activation(
                    st[:, nsl], st[:, nsl], mybir.ActivationFunctionType.Tanh
                )
                nc.vector.tensor_reduce(
                    macc[:, n * GP:(n + 1) * GP],
                    st[:, nsl].rearrange("p (g s) -> p g s", s=STATES),
                    axis=mybir.AxisListType.X,
                    op=mybir.AluOpType.add,
                )
            nc.scalar.dma_start(state[msl, :], st[:])

            mout = mpool.tile([P, HID_LOC], f32)
            nc.vector.tensor_scalar_mul(mout[:], macc[:], 1.0 / STATES)
            nc.gpsimd.dma_start(mean[msl, :], mout[:])

    nc.compile()
    return nc


def _get_nc():
    if "nc" not in _CACHE:
        _CACHE["nc"] = _build_nc()
    return _CACHE["nc"]


def _shard_inputs(x, h, W):
    N_LOC = HID * STATES // N_CORES
    xTf = np.ascontiguousarray(x.T)                       # [256, 4096]
    w2 = W.reshape(IN_DIM, HID * STATES)
    h2 = h.reshape(BATCH, HID * STATES)
    in_maps = []
    for c in range(N_CORES):
        sl = slice(c * N_LOC, (c + 1) * N_LOC)
        in_maps.append({
            "xT": xTf,
            "w": np.ascontiguousarray(w2[:, sl]),
            "h": h2[:, sl] * np.float32(0.9),
        })
    return in_maps


def kernel(x, h_quantum, W_quantum, _nc=None, _run_kwargs=None):
    from concourse.bass_utils import run_bass_kernel_spmd

    x = np.asarray(x, dtype=np.float32)
    h = np.asarray(h_quantum, dtype=np.float32)
    W = np.asarray(W_quantum, dtype=np.float32)

    nc = _nc if _nc is not None else _get_nc()
    in_maps = _shard_inputs(x, h, W)
    res = run_bass_kernel_spmd(
        nc, in_maps, core_ids=list(range(N_CORES)), **(_run_kwargs or {})
    )
    outs = res.results
    state = np.concatenate(
        [outs[c]["state"] for c in range(N_CORES)], axis=1
    ).reshape(BATCH, HID, STATES)
    mean = np.concatenate([outs[c]["mean"] for c in range(N_CORES)], axis=1)
    if _run_kwargs:
        _CACHE["last_results"] = res
    return mean.astype(np.float32, copy=False), state.astype(np.float32, copy=False)
